# revision 28
# baseline (speedup 1.0000x reference)
"""Trainium2 Bass kernel for single-head causal attention.

Problem: x[4, 4096, 100], Wq/Wk/Wv[100, 64] ->
         softmax(tril(x@Wq @ (x@Wk)^T / 8)) @ (x@Wv)   -> [4, 4096, 64]

Sharding (8 cores, SPMD single program):
  core = 2*b + c: batch b in 0..3, key-parity c in 0..1.
  Each core handles ALL 4096 queries of its batch but only the keys/values at
  global rows {c, c+2, c+4, ...} (2048 of them). This keeps the causal
  structure IDENTICAL across cores (local key tile kk is attended by global
  query columns q >= 256*kk, for both parities), so one program serves all 8
  cores; the one-column parity offset lives in a tiny [128, 256] data mask.
  Softmax is computed without max-subtraction (scores are bounded ~|s|<=9
  after the 1/8 scale, exp can't overflow), so the two half-key partials
  combine on the host as (num_A + num_B) / (den_A + den_B).

Per-core program (flash-attention style, scores kept transposed):
  qT = Wq^T x^T  [64, 4096],  kT = Wk^T xkv^T [64, 2048]  (bf16)
  V1[kk] = [x_kv@Wv | 1] per 128-key tile (bf16, ones col -> denominator)
  All PE psum products (S^T tiles, projections, warm-up) stream through a
  6-bank PSUM RING; exp runs one ACT call per contiguous run of up to 4
  S^T tiles (the ~350-cycle ACTIVATE pipe-fill overhead amortizes 4x).
  for each 512-query block qm (8 blocks):
    for each ring-contiguous group of key tiles in the strip 0..2qm+1:
      S^T[tile] = kT[tile]^T-block @ qT-block  (bf16, PSUM f32 [128,512];
          the strip's LAST tile computes only its valid 256 q-columns)
      E = exp(S^T / 8)  (one ACT call per group, PSUM->SBUF bf16)
      boundary tiles: E *= mask (DVE)
      out' += V1[kk]^T @ E  (bf16 matmuls, accumulate [65, 512] in PSUM,
          emitted one group late so the in-order PE overlaps next S^T)
    flush out' -> SBUF -> DRAM out[65, 4096]  (row 64 = sum exp = denom)

Perf notes (HW-traced):
  - PE HAM clock gate: the PE runs at 1.2 GHz until a fully-busy ~3.4us
    window passes, and the monitor watches datapath TOGGLES (all-zero
    warm-up data does NOT register). 8 dependency-free iota matmuls at
    t~8us warm the clock to 2.4 GHz under the DMA-latency head.
  - bf16 qT/kT: 128-col S^T stationary loads qualify for FWL (2x LDWEIGHTS).
  - DMA: ~0.85us serial issue cost per dma_start and ~5us issue-to-semaphore
    latency; inputs are split across the sync/scalar/gpsimd queues with the
    first-use chain (w3 | xq0 | xkv0) in parallel on three queues.
  - Projections are dribbled 1-2 per group as PE filler; clumping them
    stalls the next S^T on the in-order PE and starves ACT.
"""

import os
from contextlib import ExitStack

import numpy as np

B, T, E, H = 4, 4096, 100, 64
TK = T // 2  # keys per core
NKT = TK // 128  # 16 local key tiles
NQB = T // 512  # 8 query blocks
N_CORES = 8

_CACHE = {}


def _mask_np(c):
    """mask[i, j] = 1 if global key (2i+c) <= query col offset j else 0."""
    import ml_dtypes

    i = np.arange(128)[:, None]
    j = np.arange(256)[None, :]
    return (j >= 2 * i + c).astype(ml_dtypes.bfloat16)


def _build():
    if "nc" in _CACHE:
        return _CACHE["nc"]

    import concourse.bacc as bacc
    import concourse.tile as tile
    from concourse import mybir
    from concourse.bass import ts, ds

    f32 = mybir.dt.float32
    bf16 = mybir.dt.bfloat16
    Exp = mybir.ActivationFunctionType.Exp
    Mult = mybir.AluOpType.mult

    nc = bacc.Bacc("TRN2", target_bir_lowering=False, debug=False,
                   num_devices=N_CORES)

    xq_d = nc.dram_tensor("xq", [E, T], bf16, kind="ExternalInput").ap()
    xkv_d = nc.dram_tensor("xkv", [E, TK], bf16, kind="ExternalInput").ap()
    w3_d = nc.dram_tensor("w3", [E, 3 * H], bf16, kind="ExternalInput").ap()
    mask_d = nc.dram_tensor("mask", [128, 256], bf16,
                            kind="ExternalInput").ap()
    out_d = nc.dram_tensor("out", [H + 1, T], f32, kind="ExternalOutput").ap()

    with tile.TileContext(nc) as tc, ExitStack() as ctx:
        sb = ctx.enter_context(tc.tile_pool(name="sb", bufs=1))
        ob_p = ctx.enter_context(tc.tile_pool(name="ob", bufs=2))
        # PSUM budget (8 banks): 6-bank ring + 2x[128,512] "o" accumulators.
        psA = ctx.enter_context(tc.tile_pool(name="psA", bufs=1, space="PSUM"))
        ps_o = ctx.enter_context(tc.tile_pool(name="ps_o", bufs=2, space="PSUM"))

        xq_t = sb.tile([E, T], bf16)
        xkv_t = sb.tile([E, TK], bf16)
        w3_t = sb.tile([E, 3 * H], bf16)
        mask_t = sb.tile([128, 256], bf16)
        wq_t = w3_t[:, 0:H]
        wk_t = w3_t[:, H:2 * H]
        wv_t = w3_t[:, 2 * H:3 * H]
        # qT/kT live duplicated in both partition halves (rows 0:64 ==
        # 64:128, written by two concurrent col-group matmuls) so S^T matmuls
        # for adjacent key tiles run CONCURRENTLY in the two halves of the
        # PE array. bf16 (not fp32r): the 128-col kT stationary loads then
        # qualify for FWL (2x faster LDWEIGHTS).
        qT_t = sb.tile([128, T], bf16)
        kT_t = sb.tile([128, TK], bf16)
        v1_t = sb.tile([128, NKT, H + 1], bf16)
        warm_t = sb.tile([128, 8], f32)
        wmm_t = sb.tile([128, 512], bf16)
        # the PSUM ring and its SBUF exp-output mirror
        ps6 = psA.tile([128, 6, 512], f32)
        e6 = sb.tile([128, 6, 512], bf16)

        # HAM warm-up (see module docstring).
        nc.gpsimd.iota(wmm_t, [[1, 512]], channel_multiplier=1,
                       allow_small_or_imprecise_dtypes=True)
        for _ in range(8):
            nc.tensor.matmul(ps6[:, 5], wmm_t[:, 0:128], wmm_t,
                             start=True, stop=True)

        nc.sync.dma_start(out=w3_t, in_=w3_d)
        nc.scalar.dma_start(out=xkv_t[:, 0:512], in_=xkv_d[:, 0:512])
        nc.gpsimd.dma_start(out=xq_t[:, 0:512], in_=xq_d[:, 0:512])
        nc.sync.dma_start(out=xq_t[:, 512:1024], in_=xq_d[:, 512:1024])
        nc.gpsimd.dma_start(out=xkv_t[:, 512:1024], in_=xkv_d[:, 512:1024])
        nc.sync.dma_start(out=xq_t[:, 1024:2048], in_=xq_d[:, 1024:2048])
        nc.gpsimd.dma_start(out=xkv_t[:, 1024:2048], in_=xkv_d[:, 1024:2048])
        nc.sync.dma_start(out=xq_t[:, 2048:3072], in_=xq_d[:, 2048:3072])
        nc.gpsimd.dma_start(out=mask_t, in_=mask_d)
        nc.sync.dma_start(out=xq_t[:, 3072:4096], in_=xq_d[:, 3072:4096])

        # First ACT instruction early: overlaps the ~2.7us exp-table load
        # with input DMA.
        nc.vector.memset(warm_t, 0.0)
        nc.scalar.activation(out=warm_t, in_=warm_t, func=Exp)
        nc.vector.memset(v1_t[:, :, H], 1.0)

        # ---- ring allocator over ps6/e6 slots ----
        # A ps6 slot is free once its exp (or projection copy) has been
        # EMITTED, but its e6 mirror stays live until the (one-group-late)
        # AV is emitted: S^T groups must avoid the pend group's slots.
        RP = [0]

        def take(n, blocked=()):
            start = RP[0]
            for _ in range(10):
                if start + n > 6:
                    start = 0
                hit = [s for s in range(start, start + n) if s in blocked]
                if hit:
                    start = max(hit) + 1
                    continue
                RP[0] = start + n
                return start
            return None

        # ---- lazy projections (each takes one ring slot) ----
        qT_done = [False] * (T // 512)
        kT_done = [False] * (TK // 512)
        v_done = [False] * NKT

        def need_qT(j):
            if qT_done[j]:
                return
            qT_done[j] = True
            ps = ps6[:, take(1)]
            nc.tensor.matmul(ps[:H], wq_t, xq_t[:, ts(j, 512)],
                             start=True, stop=True)
            nc.tensor.matmul(ps[H:128], wq_t, xq_t[:, ts(j, 512)],
                             start=True, stop=True, tile_position=(0, 64))
            nc.vector.tensor_copy(qT_t[:, ts(j, 512)], ps)

        def need_kT(j):
            if kT_done[j]:
                return
            kT_done[j] = True
            ps = ps6[:, take(1)]
            nc.tensor.matmul(ps[:H], wk_t, xkv_t[:, ts(j, 512)],
                             start=True, stop=True)
            nc.tensor.matmul(ps[H:128], wk_t, xkv_t[:, ts(j, 512)],
                             start=True, stop=True, tile_position=(0, 64))
            nc.vector.tensor_copy(kT_t[:, ts(j, 512)], ps)

        def need_v(kk):
            if v_done[kk]:
                return
            v_done[kk] = True
            ps = ps6[:, take(1)]
            nc.tensor.matmul(ps[:, :H], xkv_t[:, ts(kk, 128)], wv_t,
                             start=True, stop=True)
            nc.vector.tensor_copy(v1_t[:, kk, :H], ps[:, :H])

        def emit_needs(q):
            n = 2 * q + 2
            need_qT(q)
            for j2 in range((n - 1) // 4 + 1):
                need_kT(j2)

        from collections import deque
        proj_q = deque()

        # ---- main attention loop ----
        def emit_av(p):
            slots, tiles, o_t, nkk, qm, packed = p
            for sl, kk in zip(slots, tiles):
                pk = packed and kk == nkk - 1
                if pk:
                    nc.tensor.matmul(o_t[:H + 1, 256:512], v1_t[:, kk],
                                     e6[:, sl, 0:256],
                                     start=False, stop=True)
                else:
                    nc.tensor.matmul(o_t[:H + 1], v1_t[:, kk], e6[:, sl],
                                     start=(kk == 0),
                                     stop=(kk == nkk - 1))
            if tiles[-1] == nkk - 1:  # last group of qm: flush out'
                ob = ob_p.tile([H + 1, 512], f32, tag="ob")
                nc.vector.tensor_copy(ob, o_t[:H + 1])
                nc.sync.dma_start(out=out_d[:, ds(512 * qm, 512)], in_=ob)

        pend = None
        for qm in range(NQB):
            nkk = 2 * qm + 2
            emit_needs(qm)  # normally a no-op (dribbled out earlier)
            if qm + 1 < NQB:
                nq = qm + 1
                proj_q.append(lambda q=nq: need_qT(q))
                jb = (2 * nq + 1) // 4
                if not kT_done[jb]:
                    proj_q.append(lambda j=jb: need_kT(j))
            packed = qm >= 1  # strip's last tile computes only 256 q-cols
            o_t = ps_o.tile([128, 512], f32, tag="o")
            qs_lo = qT_t[:H, ds(512 * qm, 512)]
            qs_hi = qT_t[H:128, ds(512 * qm, 512)]
            pos = 0
            while pos < nkk:
                blocked = set(pend[0]) if pend is not None else set()
                gw = min(4, nkk - pos)
                if qm == NQB - 1 and pos < nkk - 1:
                    # shortest-possible final chain: last tile of the last
                    # strip exps alone (256 cols)
                    gw = min(gw, nkk - 1 - pos)
                s0 = None
                for g in range(gw, 0, -1):
                    s0 = take(g, blocked)
                    if s0 is not None:
                        break
                tiles = list(range(pos, pos + g))
                pos += g
                slots = list(range(s0, s0 + g))
                cols = 0  # valid flattened cols in this group
                for sl, kk in zip(slots, tiles):
                    half = kk % 2
                    kts = kT_t[:H, ts(kk, 128)] if half == 0 \
                        else kT_t[H:128, ts(kk, 128)]
                    qs = qs_lo if half == 0 else qs_hi
                    if packed and kk == nkk - 1:
                        nc.tensor.matmul(ps6[:, sl, 0:256], kts,
                                         qs[:, 256:512],
                                         start=True, stop=True)
                        cols += 256
                    else:
                        nc.tensor.matmul(ps6[:, sl], kts, qs,
                                         start=True, stop=True)
                        cols += 512
                sf = ps6.rearrange("p a b -> p (a b)")
                ef = e6.rearrange("p a b -> p (a b)")
                nc.scalar.activation(out=ef[:, 512 * s0:512 * s0 + cols],
                                     in_=sf[:, 512 * s0:512 * s0 + cols],
                                     func=Exp, scale=float(H) ** -0.5)
                if proj_q:
                    proj_q.popleft()()
                if qm >= 2 and proj_q:
                    proj_q.popleft()()
                for sl, kk in zip(slots, tiles):
                    # boundary masking: tile kk borders the causal diagonal
                    # at query cols [256*kk - 512*qm, +256)
                    lo = 256 * kk - 512 * qm
                    if 0 <= lo < 512:
                        co = 0 if (packed and kk == nkk - 1) else lo
                        nc.vector.tensor_tensor(e6[:, sl, co:co + 256],
                                                e6[:, sl, co:co + 256],
                                                mask_t, Mult)
                    if qm == 0 and kk == 1:
                        # strip 0 is unpacked: q cols [0:256) of tile 1 are
                        # entirely below the diagonal
                        nc.vector.memset(e6[:, sl, 0:256], 0.0)
                # V projections ride as PE filler; consumed one group later.
                for kk in tiles:
                    need_v(kk)
                if pend is not None:
                    emit_av(pend)
                pend = (slots, tiles, o_t, nkk, qm, packed)
        emit_av(pend)

    nc.compile()
    _CACHE["nc"] = nc
    return nc


def _bf16(a):
    import ml_dtypes

    return np.ascontiguousarray(a, dtype=np.float32).astype(ml_dtypes.bfloat16)


def _make_in_maps(x, Wq, Wk, Wv):
    import ml_dtypes

    x = np.asarray(x, dtype=np.float32)
    w3 = np.zeros((E, 3 * H), dtype=ml_dtypes.bfloat16)
    w3[:, 0:H] = _bf16(Wq)
    w3[:, H:2 * H] = _bf16(Wk)
    w3[:, 2 * H:3 * H] = _bf16(Wv)
    masks = [_mask_np(0), _mask_np(1)]
    in_maps = []
    for core in range(N_CORES):
        b, c = divmod(core, 2)
        in_maps.append({
            "xq": _bf16(x[b].T),
            "xkv": _bf16(x[b, c::2, :].T),
            "w3": w3,
            "mask": masks[c],
        })
    return in_maps


def _combine(results):
    out = np.empty((B, T, H), dtype=np.float32)
    for b in range(B):
        a = results[2 * b]["out"]
        bb = results[2 * b + 1]["out"]
        num = a[:H] + bb[:H]
        den = a[H] + bb[H]
        out[b] = (num / den).T
    return out


def run(x, Wq, Wk, Wv, trace=False):
    """Returns (output [4,4096,64] f32, exec_time_ns or None)."""
    from concourse.bass_utils import run_bass_kernel_spmd

    nc = _build()
    in_maps = _make_in_maps(x, Wq, Wk, Wv)
    res = run_bass_kernel_spmd(nc, in_maps, core_ids=list(range(N_CORES)),
                               trace=trace)
    return _combine(res.results), res


def kernel(x, Wq, Wk, Wv):
    out, _ = run(x, Wq, Wk, Wv, trace=False)
    return out
